# revision 4
# baseline (speedup 1.0000x reference)
"""Multi-head attention (8 heads, d_model=384) on 8 trn2 NeuronCores.

Sharding: data-parallel over batch (B=8 -> one batch element per core).

Per-core device kernel (all f32):
  - inputs are host-side transposed+augmented: xT_aug [512, S] holds x.T in
    rows 0:384, ones in row 384 (bias fusion), zeros elsewhere; weights are
    padded head-major [512, 512]: head h occupies output cols h*64..h*64+48
    (softmax scale folded into Wq; Wv additionally carries a fused ones
    column at h*64+48 so P@V also produces softmax denominators).
  - projections give qT/kT as [d_out_pad, S] tiles (heads at 64-row offsets
    inside 128-partition tiles) and v as natural [S, d_out_pad] tiles.
  - scores are computed transposed per head: S^T[sk, sq] = kT.T-slice @ qT,
    so softmax runs along partitions and P@V consumes P^T directly with v
    stationary (no transposes anywhere on the hot path).
  - exp on ACT straight out of PSUM; P^T@... accumulation in PSUM over the
    16 sk stripes; row 48 of the result is the softmax denominator.
  - denominator row is broadcast to 128 partitions with a ones[1,128]
    matmul, reciprocal on DVE, then stripes are normalized (DVE/GPSIMD) and
    DMA'd to attnT[h, sk, sq]. Host returns a transposed view.
"""
import sys

sys.path.insert(0, "/opt/trn_rl_repo")

import numpy as np

import concourse.bass as bass
import concourse.mybir as mybir
import concourse.tile as tile
from concourse import bacc
from concourse.bass_utils import run_bass_kernel_spmd

F32 = mybir.dt.float32

B = 8
D = 384
H = 8
DH = 48
HP = 64          # padded per-head width in the q/k head-major layout
DP = H * HP      # 512: padded d_out for q/k
HPV = 72         # per-head stride in the v layout (ones col at +64, 32-aligned)
DPV = H * HPV    # 576: padded d_out for v
DIN = 512        # augmented+padded d_in (384 data + 1 ones + pad)
KC = DIN // 128  # k-chunks for projections


def build_nc(S: int, BQ: int, stripe_bufs: int = 18):
    """Build the per-core Bass program. S = sequence length, BQ = sq block."""
    NSK = S // 128           # sk stripes
    NB = S // BQ             # sq blocks
    NQC = BQ // 512          # 512-wide matmul chunks per block
    assert BQ % 512 == 0 and S % BQ == 0 and S % 512 == 0

    nc = bacc.Bacc("TRN2", target_bir_lowering=False, debug=False)
    xq = nc.dram_tensor("xq", [DIN, S], F32, kind="ExternalInput")
    xk = nc.dram_tensor("xk", [DIN, S], F32, kind="ExternalInput")
    xv = nc.dram_tensor("xv", [DIN, S], F32, kind="ExternalInput")
    wq = nc.dram_tensor("wq", [DIN, DP], F32, kind="ExternalInput")
    wk = nc.dram_tensor("wk", [DIN, DP], F32, kind="ExternalInput")
    wv = nc.dram_tensor("wv", [DIN, DPV], F32, kind="ExternalInput")
    attnT = nc.dram_tensor("attnT", [H, S, S], F32, kind="ExternalOutput")
    ctxT = nc.dram_tensor("ctxT", [D, S], F32, kind="ExternalOutput")

    with tile.TileContext(nc) as tc:
        with tc.tile_pool(name="persist", bufs=1) as persist:
            qT = [persist.tile([128, S], F32, tag=f"qT{i}", name=f"qT{i}") for i in range(4)]
            kT = [persist.tile([128, S], F32, tag=f"kT{i}", name=f"kT{i}") for i in range(4)]
            vp = [persist.tile([128, DPV], F32, tag=f"vp{i}", name=f"vp{i}") for i in range(S // 128)]
            ones = persist.tile([1, 128], F32, tag="ones")
            nc.vector.memset(ones, 1.0)

            # ---- projections ----
            with (
                tc.tile_pool(name="xin", bufs=2 * KC + 1) as xpool,
                tc.tile_pool(name="wpool", bufs=1) as wpool,
                tc.tile_pool(name="pj", bufs=3, space="PSUM") as pjpool,
            ):
                w_sb = {}
                for name, wdram in (("q", wq), ("k", wk), ("v", wv)):
                    wid = DPV if name == "v" else DP
                    w_sb[name] = [
                        wpool.tile([128, wid], F32, tag=f"w{name}{i}", name=f"w{name}{i}") for i in range(KC)
                    ]
                    for i in range(KC):
                        nc.sync.dma_start(
                            out=w_sb[name][i], in_=wdram[i * 128:(i + 1) * 128, :]
                        )

                # qT / kT: weights stationary, activations streamed
                for name, xdram, outT in (("q", xq, qT), ("k", xk, kT)):
                    xt = [xpool.tile([128, S], F32, tag="x", name="x") for _ in range(KC)]
                    for i in range(KC):
                        nc.sync.dma_start(out=xt[i], in_=xdram[i * 128:(i + 1) * 128, :])
                    for m in range(4):
                        for n in range(S // 512):
                            ps = pjpool.tile([128, 512], F32, tag="pj")
                            for kc in range(KC):
                                nc.tensor.matmul(
                                    ps,
                                    lhsT=w_sb[name][kc][:, m * 128:(m + 1) * 128],
                                    rhs=xt[kc][:, n * 512:(n + 1) * 512],
                                    start=(kc == 0),
                                    stop=(kc == KC - 1),
                                )
                            nc.vector.tensor_copy(outT[m][:, n * 512:(n + 1) * 512], ps)

                # v: activations stationary, weights streamed -> natural layout
                xt = [xpool.tile([128, S], F32, tag="x", name="x") for _ in range(KC)]
                for i in range(KC):
                    nc.sync.dma_start(out=xt[i], in_=xv[i * 128:(i + 1) * 128, :])
                vchunks = [(0, 512), (512, DPV - 512)]
                for mt in range(S // 128):
                    ps = pjpool.tile([128, DPV], F32, tag="pj", name="psv")
                    for c0, cw in vchunks:
                        for kc in range(KC):
                            nc.tensor.matmul(
                                ps[:, c0:c0 + cw],
                                lhsT=xt[kc][:, mt * 128:(mt + 1) * 128],
                                rhs=w_sb["v"][kc][:, c0:c0 + cw],
                                start=(kc == 0),
                                stop=(kc == KC - 1),
                            )
                    nc.scalar.copy(vp[mt], ps)

            # ---- attention heads ----
            with (
                tc.tile_pool(name="stripes", bufs=stripe_bufs) as stp,
                tc.tile_pool(name="misc", bufs=2) as misc,
                tc.tile_pool(name="spsum", bufs=2, space="PSUM") as sps_pool,
                tc.tile_pool(name="cpsum", bufs=2, space="PSUM") as cps_pool,
            ):
                for h in range(H):
                    t, off = h // 2, (h % 2) * 64
                    for blk in range(NB):
                        q0 = blk * BQ
                        cps = cps_pool.tile([65, BQ], F32, tag="c")
                        pts = []
                        for j in range(NSK):
                            sps = sps_pool.tile([128, BQ], F32, tag="s")
                            for n in range(NQC):
                                nc.tensor.matmul(
                                    sps[:, n * 512:(n + 1) * 512],
                                    lhsT=kT[t][off:off + DH, j * 128:(j + 1) * 128],
                                    rhs=qT[t][off:off + DH, q0 + n * 512:q0 + (n + 1) * 512],
                                    start=True,
                                    stop=True,
                                )
                            pt = stp.tile([128, BQ], F32, tag="pt")
                            nc.scalar.activation(
                                pt, sps, mybir.ActivationFunctionType.Exp
                            )
                            for n in range(NQC):
                                nc.tensor.matmul(
                                    cps[0:65, n * 512:(n + 1) * 512],
                                    lhsT=vp[j][:, h * HPV:h * HPV + 65],
                                    rhs=pt[:, n * 512:(n + 1) * 512],
                                    start=(j == 0),
                                    stop=(j == NSK - 1),
                                )
                            pts.append(pt)
                        # denominators -> broadcast -> reciprocal
                        drow = misc.tile([1, BQ], F32, tag="drow")
                        nc.scalar.copy(drow, cps[64:65, :])
                        bps = sps_pool.tile([128, BQ], F32, tag="s")
                        for n in range(NQC):
                            nc.tensor.matmul(
                                bps[:, n * 512:(n + 1) * 512],
                                lhsT=ones,
                                rhs=drow[:, n * 512:(n + 1) * 512],
                                start=True,
                                stop=True,
                            )
                        rb = misc.tile([128, BQ], F32, tag="rb")
                        nc.vector.reciprocal(rb, bps)
                        # context slice for this (head, block)
                        ctx = misc.tile([DH, BQ], F32, tag="ctx")
                        nc.vector.tensor_mul(ctx, cps[0:DH, :], rb[0:DH, :])
                        nc.sync.dma_start(
                            out=ctxT[h * DH:(h + 1) * DH, q0:q0 + BQ], in_=ctx
                        )
                        # normalize + write out the attention stripes
                        for j in range(NSK):
                            eng = nc.gpsimd if j % 3 == 2 else nc.vector
                            eng.tensor_mul(pts[j], pts[j], rb)
                            nc.sync.dma_start(
                                out=attnT[h, j * 128:(j + 1) * 128, q0:q0 + BQ],
                                in_=pts[j],
                            )
    nc.finalize()
    return nc


def _prep_weights(Wq, bq, Wk, bk, Wv, bv):
    """Padded head-major, bias-augmented weight mats [DIN, DP]."""
    scale = np.float32(1.0 / np.sqrt(np.float32(DH)))
    wqp = np.zeros((DIN, DP), np.float32)
    wkp = np.zeros((DIN, DP), np.float32)
    wvp = np.zeros((DIN, DPV), np.float32)
    for h in range(H):
        src = slice(h * DH, (h + 1) * DH)
        dst = slice(h * HP, h * HP + DH)
        dstv = slice(h * HPV, h * HPV + DH)
        wqp[0:D, dst] = Wq[:, src] * scale
        wqp[D, dst] = bq[src] * scale
        wkp[0:D, dst] = Wk[:, src]
        wkp[D, dst] = bk[src]
        wvp[0:D, dstv] = Wv[:, src]
        wvp[D, dstv] = bv[src]
        wvp[D, h * HPV + 64] = 1.0  # fused ones column -> softmax denominators
    return wqp, wkp, wvp


def _prep_x(x):
    """[S, D] -> transposed+augmented [DIN, S]."""
    S = x.shape[0]
    xa = np.zeros((DIN, S), np.float32)
    xa[0:D] = x.T
    xa[D] = 1.0
    return xa


def run_attention(query, key, value, Wq, bq, Wk, bk, Wv, bv, S, BQ, trace=False,
                  **run_kwargs):
    nb = query.shape[0]
    nc = build_nc(S, BQ)
    wqp, wkp, wvp = _prep_weights(Wq, bq, Wk, bk, Wv, bv)
    in_maps = []
    for b in range(nb):
        in_maps.append(
            {
                "xq": _prep_x(query[b]),
                "xk": _prep_x(key[b]),
                "xv": _prep_x(value[b]),
                "wq": wqp,
                "wk": wkp,
                "wv": wvp,
            }
        )
    res = run_bass_kernel_spmd(
        nc, in_maps, core_ids=list(range(nb)), trace=trace, **run_kwargs
    )
    # gather: context[b] = ctxT_b.T ; attn[h*B+b, sq, sk] = attnT_b[h, sk, sq]
    context = np.stack([res.results[b]["ctxT"].T for b in range(nb)], axis=0)
    attn = (
        np.stack([res.results[b]["attnT"] for b in range(nb)], axis=1)
        .reshape(H * nb, S, S)
        .swapaxes(1, 2)
    )
    return (context, attn), res


def kernel(query, key, value, Wq, bq, Wk, bk, Wv, bv):
    query = np.asarray(query, np.float32)
    key = np.asarray(key, np.float32)
    value = np.asarray(value, np.float32)
    Wq = np.asarray(Wq, np.float32)
    bq = np.asarray(bq, np.float32)
    Wk = np.asarray(Wk, np.float32)
    bk = np.asarray(bk, np.float32)
    Wv = np.asarray(Wv, np.float32)
    bv = np.asarray(bv, np.float32)
    S = query.shape[1]
    out, _ = run_attention(
        query, key, value, Wq, bq, Wk, bk, Wv, bv, S=S, BQ=1024
    )
    return out


# revision 5
# speedup vs baseline: 1.9297x; 1.9297x over previous
"""Multi-head attention (8 heads, d_model=384) on 8 trn2 NeuronCores.

Sharding: data-parallel over batch (B=8 -> one batch element per core).

Per-core device kernel (all f32):
  - inputs are host-side transposed+augmented: xT_aug [512, S] holds x.T in
    rows 0:384, ones in row 384 (bias fusion), zeros elsewhere; weights are
    padded head-major [512, 512]: head h occupies output cols h*64..h*64+48
    (softmax scale folded into Wq; Wv additionally carries a fused ones
    column at h*64+48 so P@V also produces softmax denominators).
  - projections give qT/kT as [d_out_pad, S] tiles (heads at 64-row offsets
    inside 128-partition tiles) and v as natural [S, d_out_pad] tiles.
  - scores are computed transposed per head: S^T[sk, sq] = kT.T-slice @ qT,
    so softmax runs along partitions and P@V consumes P^T directly with v
    stationary (no transposes anywhere on the hot path).
  - exp on ACT straight out of PSUM; P^T@... accumulation in PSUM over the
    16 sk stripes; row 48 of the result is the softmax denominator.
  - denominator row is broadcast to 128 partitions with a ones[1,128]
    matmul, reciprocal on DVE, then stripes are normalized (DVE/GPSIMD) and
    DMA'd to attnT[h, sk, sq]. Host returns a transposed view.
"""
import sys

sys.path.insert(0, "/opt/trn_rl_repo")

import numpy as np

import concourse.bass as bass
import concourse.mybir as mybir
import concourse.tile as tile
from concourse import bacc
from concourse.bass_utils import run_bass_kernel_spmd

F32 = mybir.dt.float32
F32R = mybir.dt.float32r
F16 = mybir.dt.float16

B = 8
D = 384
H = 8
DH = 48
HP = 64          # padded per-head width in the q/k head-major layout
DP = H * HP      # 512: padded d_out for q/k
HPV = 72         # per-head stride in the v layout (ones col at +64, 32-aligned)
DPV = H * HPV    # 576: padded d_out for v
DIN = 512        # augmented+padded d_in (384 data + 1 ones + pad)
KC = DIN // 128  # k-chunks for projections


def build_nc(S: int, BQ: int, stripe_bufs: int = 24):
    """Build the per-core Bass program. S = sequence length, BQ = sq block."""
    NSK = S // 128           # sk stripes
    NB = S // BQ             # sq blocks
    NQC = BQ // 512          # 512-wide matmul chunks per block
    assert BQ % 512 == 0 and S % BQ == 0 and S % 512 == 0

    nc = bacc.Bacc("TRN2", target_bir_lowering=False, debug=False)
    xq = nc.dram_tensor("xq", [DIN, S], F32R, kind="ExternalInput")
    xk = nc.dram_tensor("xk", [DIN, S], F32R, kind="ExternalInput")
    xv = nc.dram_tensor("xv", [DIN, S], F32R, kind="ExternalInput")
    wq = nc.dram_tensor("wq", [DIN, DP], F32R, kind="ExternalInput")
    wk = nc.dram_tensor("wk", [DIN, DP], F32R, kind="ExternalInput")
    wv = nc.dram_tensor("wv", [DIN, DPV], F32R, kind="ExternalInput")
    attnT = nc.dram_tensor("attnT", [H, S, S], F16, kind="ExternalOutput")
    ctxT = nc.dram_tensor("ctxT", [D, S], F32, kind="ExternalOutput")

    with tile.TileContext(nc) as tc:
        with tc.tile_pool(name="persist", bufs=1) as persist:
            qT = [persist.tile([128, S], F32R, tag=f"qT{i}", name=f"qT{i}") for i in range(4)]
            kT = [persist.tile([128, S], F32R, tag=f"kT{i}", name=f"kT{i}") for i in range(4)]
            vp = [persist.tile([128, DPV], F16, tag=f"vp{i}", name=f"vp{i}") for i in range(S // 128)]
            ones = persist.tile([1, 128], F16, tag="ones")
            nc.vector.memset(ones, 1.0)

            # ---- projections ----
            with (
                tc.tile_pool(name="xin", bufs=2 * KC + 1) as xpool,
                tc.tile_pool(name="wpool", bufs=1) as wpool,
                tc.tile_pool(name="pj", bufs=3, space="PSUM") as pjpool,
            ):
                w_sb = {}
                for name, wdram in (("q", wq), ("k", wk), ("v", wv)):
                    wid = DPV if name == "v" else DP
                    w_sb[name] = [
                        wpool.tile([128, wid], F32R, tag=f"w{name}{i}", name=f"w{name}{i}") for i in range(KC)
                    ]
                    for i in range(KC):
                        nc.sync.dma_start(
                            out=w_sb[name][i], in_=wdram[i * 128:(i + 1) * 128, :]
                        )

                # qT / kT: weights stationary, activations streamed
                for name, xdram, outT in (("q", xq, qT), ("k", xk, kT)):
                    xt = [xpool.tile([128, S], F32R, tag="x", name="x") for _ in range(KC)]
                    for i in range(KC):
                        nc.sync.dma_start(out=xt[i], in_=xdram[i * 128:(i + 1) * 128, :])
                    for m in range(4):
                        for n in range(S // 512):
                            ps = pjpool.tile([128, 512], F32, tag="pj")
                            for kc in range(KC):
                                nc.tensor.matmul(
                                    ps,
                                    lhsT=w_sb[name][kc][:, m * 128:(m + 1) * 128],
                                    rhs=xt[kc][:, n * 512:(n + 1) * 512],
                                    start=(kc == 0),
                                    stop=(kc == KC - 1),
                                )
                            nc.vector.tensor_copy(outT[m][:, n * 512:(n + 1) * 512], ps)

                # v: activations stationary, weights streamed -> natural layout
                xt = [xpool.tile([128, S], F32R, tag="x", name="x") for _ in range(KC)]
                for i in range(KC):
                    nc.sync.dma_start(out=xt[i], in_=xv[i * 128:(i + 1) * 128, :])
                vchunks = [(0, 512), (512, DPV - 512)]
                for mt in range(S // 128):
                    ps = pjpool.tile([128, DPV], F32, tag="pj", name="psv")
                    for c0, cw in vchunks:
                        for kc in range(KC):
                            nc.tensor.matmul(
                                ps[:, c0:c0 + cw],
                                lhsT=xt[kc][:, mt * 128:(mt + 1) * 128],
                                rhs=w_sb["v"][kc][:, c0:c0 + cw],
                                start=(kc == 0),
                                stop=(kc == KC - 1),
                            )
                    nc.scalar.copy(vp[mt], ps)

            # ---- attention heads ----
            with (
                tc.tile_pool(name="stripes", bufs=stripe_bufs) as stp,
                tc.tile_pool(name="misc", bufs=2) as misc,
                tc.tile_pool(name="spsum", bufs=2, space="PSUM") as sps_pool,
                tc.tile_pool(name="cpsum", bufs=2, space="PSUM") as cps_pool,
            ):
                for h in range(H):
                    t, off = h // 2, (h % 2) * 64
                    for blk in range(NB):
                        q0 = blk * BQ
                        cps = cps_pool.tile([65, BQ], F32, tag="c")
                        pts = []
                        for j in range(NSK):
                            sps = sps_pool.tile([128, BQ], F32, tag="s")
                            for n in range(NQC):
                                nc.tensor.matmul(
                                    sps[:, n * 512:(n + 1) * 512],
                                    lhsT=kT[t][off:off + DH, j * 128:(j + 1) * 128],
                                    rhs=qT[t][off:off + DH, q0 + n * 512:q0 + (n + 1) * 512],
                                    start=True,
                                    stop=True,
                                )
                            pt = stp.tile([128, BQ], F16, tag="pt")
                            nc.scalar.activation(
                                pt, sps, mybir.ActivationFunctionType.Exp
                            )
                            for n in range(NQC):
                                nc.tensor.matmul(
                                    cps[0:65, n * 512:(n + 1) * 512],
                                    lhsT=vp[j][:, h * HPV:h * HPV + 65],
                                    rhs=pt[:, n * 512:(n + 1) * 512],
                                    start=(j == 0),
                                    stop=(j == NSK - 1),
                                )
                            pts.append(pt)
                        # denominators -> broadcast -> reciprocal
                        drow = misc.tile([1, BQ], F16, tag="drow")
                        nc.scalar.copy(drow, cps[64:65, :])
                        bps = sps_pool.tile([128, BQ], F32, tag="s")
                        for n in range(NQC):
                            nc.tensor.matmul(
                                bps[:, n * 512:(n + 1) * 512],
                                lhsT=ones,
                                rhs=drow[:, n * 512:(n + 1) * 512],
                                start=True,
                                stop=True,
                            )
                        rb = misc.tile([128, BQ], F32, tag="rb")
                        nc.vector.reciprocal(rb, bps)
                        rb16 = misc.tile([128, BQ], F16, tag="rb16")
                        nc.vector.tensor_copy(rb16, rb)
                        # context slice for this (head, block)
                        ctx = misc.tile([DH, BQ], F32, tag="ctx")
                        nc.vector.tensor_mul(ctx, cps[0:DH, :], rb[0:DH, :])
                        nc.sync.dma_start(
                            out=ctxT[h * DH:(h + 1) * DH, q0:q0 + BQ], in_=ctx
                        )
                        # normalize + write out the attention stripes
                        for j in range(NSK):
                            eng = nc.gpsimd if j % 8 == 7 else nc.vector
                            eng.tensor_mul(pts[j], pts[j], rb16)
                            nc.sync.dma_start(
                                out=attnT[h, j * 128:(j + 1) * 128, q0:q0 + BQ],
                                in_=pts[j],
                            )
    nc.finalize()
    return nc


def _prep_weights(Wq, bq, Wk, bk, Wv, bv):
    """Padded head-major, bias-augmented weight mats [DIN, DP]."""
    scale = np.float32(1.0 / np.sqrt(np.float32(DH)))
    wqp = np.zeros((DIN, DP), np.float32)
    wkp = np.zeros((DIN, DP), np.float32)
    wvp = np.zeros((DIN, DPV), np.float32)
    for h in range(H):
        src = slice(h * DH, (h + 1) * DH)
        dst = slice(h * HP, h * HP + DH)
        dstv = slice(h * HPV, h * HPV + DH)
        wqp[0:D, dst] = Wq[:, src] * scale
        wqp[D, dst] = bq[src] * scale
        wkp[0:D, dst] = Wk[:, src]
        wkp[D, dst] = bk[src]
        wvp[0:D, dstv] = Wv[:, src]
        wvp[D, dstv] = bv[src]
        wvp[D, h * HPV + 64] = 1.0  # fused ones column -> softmax denominators
    return wqp, wkp, wvp


def _prep_x(x):
    """[S, D] -> transposed+augmented [DIN, S]."""
    S = x.shape[0]
    xa = np.zeros((DIN, S), np.float32)
    xa[0:D] = x.T
    xa[D] = 1.0
    return xa


def run_attention(query, key, value, Wq, bq, Wk, bk, Wv, bv, S, BQ, trace=False,
                  **run_kwargs):
    nb = query.shape[0]
    nc = build_nc(S, BQ)
    wqp, wkp, wvp = _prep_weights(Wq, bq, Wk, bk, Wv, bv)
    in_maps = []
    for b in range(nb):
        in_maps.append(
            {
                "xq": _prep_x(query[b]),
                "xk": _prep_x(key[b]),
                "xv": _prep_x(value[b]),
                "wq": wqp,
                "wk": wkp,
                "wv": wvp,
            }
        )
    res = run_bass_kernel_spmd(
        nc, in_maps, core_ids=list(range(nb)), trace=trace, **run_kwargs
    )
    # gather: context[b] = ctxT_b.T ; attn[h*B+b, sq, sk] = attnT_b[h, sk, sq]
    context = np.stack([res.results[b]["ctxT"].T for b in range(nb)], axis=0)
    attn = (
        np.stack([res.results[b]["attnT"] for b in range(nb)], axis=1)
        .astype(np.float32)
        .reshape(H * nb, S, S)
        .swapaxes(1, 2)
    )
    return (context, attn), res


def kernel(query, key, value, Wq, bq, Wk, bk, Wv, bv):
    query = np.asarray(query, np.float32)
    key = np.asarray(key, np.float32)
    value = np.asarray(value, np.float32)
    Wq = np.asarray(Wq, np.float32)
    bq = np.asarray(bq, np.float32)
    Wk = np.asarray(Wk, np.float32)
    bk = np.asarray(bk, np.float32)
    Wv = np.asarray(Wv, np.float32)
    bv = np.asarray(bv, np.float32)
    S = query.shape[1]
    out, _ = run_attention(
        query, key, value, Wq, bq, Wk, bk, Wv, bv, S=S, BQ=1024
    )
    return out


# revision 7
# speedup vs baseline: 2.4986x; 1.2948x over previous
"""Multi-head attention (8 heads, d_model=384) on 8 trn2 NeuronCores.

Sharding: data-parallel over batch (B=8 -> one batch element per core).

Per-core device kernel (all f32):
  - inputs are host-side transposed+augmented: xT_aug [512, S] holds x.T in
    rows 0:384, ones in row 384 (bias fusion), zeros elsewhere; weights are
    padded head-major [512, 512]: head h occupies output cols h*64..h*64+48
    (softmax scale folded into Wq; Wv additionally carries a fused ones
    column at h*64+48 so P@V also produces softmax denominators).
  - projections give qT/kT as [d_out_pad, S] tiles (heads at 64-row offsets
    inside 128-partition tiles) and v as natural [S, d_out_pad] tiles.
  - scores are computed transposed per head: S^T[sk, sq] = kT.T-slice @ qT,
    so softmax runs along partitions and P@V consumes P^T directly with v
    stationary (no transposes anywhere on the hot path).
  - exp on ACT straight out of PSUM; P^T@... accumulation in PSUM over the
    16 sk stripes; row 48 of the result is the softmax denominator.
  - denominator row is broadcast to 128 partitions with a ones[1,128]
    matmul, reciprocal on DVE, then stripes are normalized (DVE/GPSIMD) and
    DMA'd to attnT[h, sk, sq]. Host returns a transposed view.
"""
import sys

sys.path.insert(0, "/opt/trn_rl_repo")

import numpy as np

import concourse.bass as bass
import concourse.mybir as mybir
import concourse.tile as tile
from concourse import bacc
from concourse.bass_utils import run_bass_kernel_spmd

F32 = mybir.dt.float32
F32R = mybir.dt.float32r
F16 = mybir.dt.float16

B = 8
D = 384
H = 8
DH = 48
HP = 64          # padded per-head width in the q/k head-major layout
DP = H * HP      # 512: padded d_out for q/k
HPV = 72         # per-head stride in the v layout (ones col at +64, 32-aligned)
DPV = H * HPV    # 576: padded d_out for v
DIN = 512        # augmented+padded d_in (384 data + 1 ones + pad)
KC = DIN // 128  # k-chunks for projections


def build_nc(S: int, BQ: int, stripe_bufs: int = 36):
    """Build the per-core Bass program. S = sequence length, BQ = sq block."""
    NSK = S // 128           # sk stripes
    NB = S // BQ             # sq blocks
    NQC = BQ // 512          # 512-wide matmul chunks per block
    assert BQ % 512 == 0 and S % BQ == 0 and S % 512 == 0

    nc = bacc.Bacc("TRN2", target_bir_lowering=False, debug=False)
    xq = nc.dram_tensor("xq", [DIN, S], F32R, kind="ExternalInput")
    xk = nc.dram_tensor("xk", [DIN, S], F32R, kind="ExternalInput")
    xv = nc.dram_tensor("xv", [DIN, S], F32R, kind="ExternalInput")
    wq = nc.dram_tensor("wq", [DIN, DP], F32R, kind="ExternalInput")
    wk = nc.dram_tensor("wk", [DIN, DP], F32R, kind="ExternalInput")
    wv = nc.dram_tensor("wv", [DIN, DPV], F32R, kind="ExternalInput")
    attnT = nc.dram_tensor("attnT", [H, S, S], F16, kind="ExternalOutput")
    ctxT = nc.dram_tensor("ctxT", [D, S], F32, kind="ExternalOutput")

    with tile.TileContext(nc) as tc:
        with tc.tile_pool(name="persist", bufs=1) as persist:
            qT = [persist.tile([128, S], F32R, tag=f"qT{i}", name=f"qT{i}") for i in range(4)]
            kT = [persist.tile([128, S], F32R, tag=f"kT{i}", name=f"kT{i}") for i in range(4)]
            vp = [persist.tile([128, DPV], F16, tag=f"vp{i}", name=f"vp{i}") for i in range(S // 128)]
            ones = persist.tile([1, 128], F16, tag="ones")
            nc.vector.memset(ones, 1.0)

            # ---- projections ----
            with (
                tc.tile_pool(name="xin", bufs=2 * KC + 1) as xpool,
                tc.tile_pool(name="wpool", bufs=1) as wpool,
                tc.tile_pool(name="pj", bufs=3, space="PSUM") as pjpool,
            ):
                w_sb = {}
                for name, wdram in (("q", wq), ("k", wk), ("v", wv)):
                    wid = DPV if name == "v" else DP
                    w_sb[name] = [
                        wpool.tile([128, wid], F32R, tag=f"w{name}{i}", name=f"w{name}{i}") for i in range(KC)
                    ]
                    for i in range(KC):
                        nc.sync.dma_start(
                            out=w_sb[name][i], in_=wdram[i * 128:(i + 1) * 128, :]
                        )

                # qT / kT: weights stationary, activations streamed
                for name, xdram, outT in (("q", xq, qT), ("k", xk, kT)):
                    xt = [xpool.tile([128, S], F32R, tag="x", name="x") for _ in range(KC)]
                    for i in range(KC):
                        nc.sync.dma_start(out=xt[i], in_=xdram[i * 128:(i + 1) * 128, :])
                    for m in range(4):
                        for n in range(S // 512):
                            ps = pjpool.tile([128, 512], F32, tag="pj")
                            for kc in range(KC):
                                nc.tensor.matmul(
                                    ps,
                                    lhsT=w_sb[name][kc][:, m * 128:(m + 1) * 128],
                                    rhs=xt[kc][:, n * 512:(n + 1) * 512],
                                    start=(kc == 0),
                                    stop=(kc == KC - 1),
                                )
                            nc.vector.tensor_copy(outT[m][:, n * 512:(n + 1) * 512], ps)

                # v: activations stationary, weights streamed -> natural layout
                xt = [xpool.tile([128, S], F32R, tag="x", name="x") for _ in range(KC)]
                for i in range(KC):
                    nc.sync.dma_start(out=xt[i], in_=xv[i * 128:(i + 1) * 128, :])
                vchunks = [(0, 512), (512, DPV - 512)]
                for mt in range(S // 128):
                    ps = pjpool.tile([128, DPV], F32, tag="pj", name="psv")
                    for c0, cw in vchunks:
                        for kc in range(KC):
                            nc.tensor.matmul(
                                ps[:, c0:c0 + cw],
                                lhsT=xt[kc][:, mt * 128:(mt + 1) * 128],
                                rhs=w_sb["v"][kc][:, c0:c0 + cw],
                                start=(kc == 0),
                                stop=(kc == KC - 1),
                            )
                    nc.scalar.copy(vp[mt], ps)

            # ---- attention heads ----
            with (
                tc.tile_pool(name="stripes", bufs=stripe_bufs) as stp,
                tc.tile_pool(name="misc", bufs=2) as misc,
                tc.tile_pool(name="spsum", bufs=2, space="PSUM") as sps_pool,
                tc.tile_pool(name="cpsum", bufs=2, space="PSUM") as cps_pool,
            ):
                # heads processed in even/odd pairs: their score matmuls use
                # base partitions 0 and 64 -> distinct PE row-groups -> the
                # two matmuls run concurrently in the array
                for hp in range(H // 2):
                    t = hp
                    heads = (2 * hp, 2 * hp + 1)
                    for blk in range(NB):
                        q0 = blk * BQ
                        cpss = [
                            cps_pool.tile([65, BQ], F32, tag="c", name="cps")
                            for _ in heads
                        ]
                        pts = {0: [], 1: []}
                        for j in range(NSK):
                            spss = [
                                sps_pool.tile([128, BQ], F32, tag="s", name="sps")
                                for _ in heads
                            ]
                            for n in range(NQC):
                                for hi in range(2):
                                    off = hi * 64
                                    nc.tensor.matmul(
                                        spss[hi][:, n * 512:(n + 1) * 512],
                                        lhsT=kT[t][off:off + DH, j * 128:(j + 1) * 128],
                                        rhs=qT[t][off:off + DH, q0 + n * 512:q0 + (n + 1) * 512],
                                        start=True,
                                        stop=True,
                                    )
                            for hi, h in enumerate(heads):
                                pt = stp.tile([128, BQ], F16, tag="pt")
                                nc.scalar.activation(
                                    pt, spss[hi], mybir.ActivationFunctionType.Exp
                                )
                                for n in range(NQC):
                                    nc.tensor.matmul(
                                        cpss[hi][0:65, n * 512:(n + 1) * 512],
                                        lhsT=vp[j][:, h * HPV:h * HPV + 65],
                                        rhs=pt[:, n * 512:(n + 1) * 512],
                                        start=(j == 0),
                                        stop=(j == NSK - 1),
                                    )
                                pts[hi].append(pt)
                        for hi, h in enumerate(heads):
                            cps = cpss[hi]
                            # denominator row -> [128, 8] scatter -> cheap
                            # reciprocal -> gather -> PE broadcast
                            drow = misc.tile([1, BQ], F32, tag="drow")
                            nc.scalar.copy(drow, cps[64:65, :])
                            dsq = misc.tile([128, BQ // 128], F32, tag="dsq")
                            nc.sync.dma_start(out=dsq, in_=drow)
                            rsq = misc.tile([128, BQ // 128], F16, tag="rsq")
                            with nc.allow_low_precision(
                                reason="f16 softmax denominators are plenty"
                            ):
                                nc.vector.reciprocal(rsq, dsq)
                            rrow = misc.tile([1, BQ], F16, tag="rrow")
                            nc.sync.dma_start(out=rrow, in_=rsq)
                            bps = sps_pool.tile([128, BQ], F32, tag="s", name="bps")
                            for n in range(NQC):
                                nc.tensor.matmul(
                                    bps[:, n * 512:(n + 1) * 512],
                                    lhsT=ones,
                                    rhs=rrow[:, n * 512:(n + 1) * 512],
                                    start=True,
                                    stop=True,
                                )
                            rb16 = misc.tile([128, BQ], F16, tag="rb16")
                            nc.vector.tensor_copy(rb16, bps)
                            # context slice for this (head, block)
                            ctx = misc.tile([DH, BQ], F32, tag="ctx")
                            nc.vector.tensor_mul(ctx, cps[0:DH, :], rb16[0:DH, :])
                            nc.sync.dma_start(
                                out=ctxT[h * DH:(h + 1) * DH, q0:q0 + BQ], in_=ctx
                            )
                            # normalize + write out the attention stripes
                            for j in range(NSK):
                                nc.vector.tensor_mul(pts[hi][j], pts[hi][j], rb16)
                                nc.sync.dma_start(
                                    out=attnT[h, j * 128:(j + 1) * 128, q0:q0 + BQ],
                                    in_=pts[hi][j],
                                )
    nc.finalize()
    return nc


def _prep_weights(Wq, bq, Wk, bk, Wv, bv):
    """Padded head-major, bias-augmented weight mats [DIN, DP]."""
    scale = np.float32(1.0 / np.sqrt(np.float32(DH)))
    wqp = np.zeros((DIN, DP), np.float32)
    wkp = np.zeros((DIN, DP), np.float32)
    wvp = np.zeros((DIN, DPV), np.float32)
    for h in range(H):
        src = slice(h * DH, (h + 1) * DH)
        dst = slice(h * HP, h * HP + DH)
        dstv = slice(h * HPV, h * HPV + DH)
        wqp[0:D, dst] = Wq[:, src] * scale
        wqp[D, dst] = bq[src] * scale
        wkp[0:D, dst] = Wk[:, src]
        wkp[D, dst] = bk[src]
        wvp[0:D, dstv] = Wv[:, src]
        wvp[D, dstv] = bv[src]
        wvp[D, h * HPV + 64] = 1.0  # fused ones column -> softmax denominators
    return wqp, wkp, wvp


def _prep_x(x):
    """[S, D] -> transposed+augmented [DIN, S]."""
    S = x.shape[0]
    xa = np.zeros((DIN, S), np.float32)
    xa[0:D] = x.T
    xa[D] = 1.0
    return xa


def run_attention(query, key, value, Wq, bq, Wk, bk, Wv, bv, S, BQ, trace=False,
                  **run_kwargs):
    nb = query.shape[0]
    nc = build_nc(S, BQ)
    wqp, wkp, wvp = _prep_weights(Wq, bq, Wk, bk, Wv, bv)
    in_maps = []
    for b in range(nb):
        in_maps.append(
            {
                "xq": _prep_x(query[b]),
                "xk": _prep_x(key[b]),
                "xv": _prep_x(value[b]),
                "wq": wqp,
                "wk": wkp,
                "wv": wvp,
            }
        )
    res = run_bass_kernel_spmd(
        nc, in_maps, core_ids=list(range(nb)), trace=trace, **run_kwargs
    )
    # gather: context[b] = ctxT_b.T ; attn[h*B+b, sq, sk] = attnT_b[h, sk, sq]
    context = np.stack([res.results[b]["ctxT"].T for b in range(nb)], axis=0)
    attn = (
        np.stack([res.results[b]["attnT"] for b in range(nb)], axis=1)
        .astype(np.float32)
        .reshape(H * nb, S, S)
        .swapaxes(1, 2)
    )
    return (context, attn), res


def kernel(query, key, value, Wq, bq, Wk, bk, Wv, bv):
    query = np.asarray(query, np.float32)
    key = np.asarray(key, np.float32)
    value = np.asarray(value, np.float32)
    Wq = np.asarray(Wq, np.float32)
    bq = np.asarray(bq, np.float32)
    Wk = np.asarray(Wk, np.float32)
    bk = np.asarray(bk, np.float32)
    Wv = np.asarray(Wv, np.float32)
    bv = np.asarray(bv, np.float32)
    S = query.shape[1]
    out, _ = run_attention(
        query, key, value, Wq, bq, Wk, bk, Wv, bv, S=S, BQ=1024
    )
    return out


# revision 8
# speedup vs baseline: 2.6416x; 1.0572x over previous
"""Multi-head attention (8 heads, d_model=384) on 8 trn2 NeuronCores.

Sharding: data-parallel over batch (B=8 -> one batch element per core).

Per-core device kernel (all f32):
  - inputs are host-side transposed+augmented: xT_aug [512, S] holds x.T in
    rows 0:384, ones in row 384 (bias fusion), zeros elsewhere; weights are
    padded head-major [512, 512]: head h occupies output cols h*64..h*64+48
    (softmax scale folded into Wq; Wv additionally carries a fused ones
    column at h*64+48 so P@V also produces softmax denominators).
  - projections give qT/kT as [d_out_pad, S] tiles (heads at 64-row offsets
    inside 128-partition tiles) and v as natural [S, d_out_pad] tiles.
  - scores are computed transposed per head: S^T[sk, sq] = kT.T-slice @ qT,
    so softmax runs along partitions and P@V consumes P^T directly with v
    stationary (no transposes anywhere on the hot path).
  - exp on ACT straight out of PSUM; P^T@... accumulation in PSUM over the
    16 sk stripes; row 48 of the result is the softmax denominator.
  - denominator row is broadcast to 128 partitions with a ones[1,128]
    matmul, reciprocal on DVE, then stripes are normalized (DVE/GPSIMD) and
    DMA'd to attnT[h, sk, sq]. Host returns a transposed view.
"""
import sys

sys.path.insert(0, "/opt/trn_rl_repo")

import numpy as np

import concourse.bass as bass
import concourse.mybir as mybir
import concourse.tile as tile
from concourse import bacc
from concourse.bass_utils import run_bass_kernel_spmd

F32 = mybir.dt.float32
F32R = mybir.dt.float32r
F16 = mybir.dt.float16

B = 8
D = 384
H = 8
DH = 48
HP = 64          # padded per-head width in the q/k head-major layout
DP = H * HP      # 512: padded d_out for q/k
HPV = 72         # per-head stride in the v layout (ones col at +64, 32-aligned)
DPV = H * HPV    # 576: padded d_out for v
DIN = 512        # augmented+padded d_in (384 data + 1 ones + pad)
KC = DIN // 128  # k-chunks for projections


def build_nc(S: int, BQ: int, stripe_bufs: int = 40):
    """Build the per-core Bass program. S = sequence length, BQ = sq block."""
    NSK = S // 128           # sk stripes
    NB = S // BQ             # sq blocks
    NQC = BQ // 512          # 512-wide matmul chunks per block
    assert BQ % 512 == 0 and S % BQ == 0 and S % 512 == 0

    nc = bacc.Bacc("TRN2", target_bir_lowering=False, debug=False)
    xq = nc.dram_tensor("xq", [DIN, S], F32R, kind="ExternalInput")
    xk = nc.dram_tensor("xk", [DIN, S], F32R, kind="ExternalInput")
    xv = nc.dram_tensor("xv", [DIN, S], F32R, kind="ExternalInput")
    wq = nc.dram_tensor("wq", [DIN, DP], F32R, kind="ExternalInput")
    wk = nc.dram_tensor("wk", [DIN, DP], F32R, kind="ExternalInput")
    wv = nc.dram_tensor("wv", [DIN, DPV], F32R, kind="ExternalInput")
    attnT = nc.dram_tensor("attnT", [H, S, S], F16, kind="ExternalOutput")
    ctxT = nc.dram_tensor("ctxT", [D, S], F32, kind="ExternalOutput")

    with tile.TileContext(nc) as tc:
        with tc.tile_pool(name="persist", bufs=1) as persist:
            qT = [persist.tile([128, S], F32R, tag=f"qT{i}", name=f"qT{i}") for i in range(4)]
            kT = [persist.tile([128, S], F32R, tag=f"kT{i}", name=f"kT{i}") for i in range(4)]
            vp = [persist.tile([128, DPV], F16, tag=f"vp{i}", name=f"vp{i}") for i in range(S // 128)]

            # ---- projections ----
            with (
                tc.tile_pool(name="xin", bufs=2 * KC + 1) as xpool,
                tc.tile_pool(name="wpool", bufs=1) as wpool,
                tc.tile_pool(name="pj", bufs=3, space="PSUM") as pjpool,
            ):
                w_sb = {}
                for name, wdram in (("q", wq), ("k", wk), ("v", wv)):
                    wid = DPV if name == "v" else DP
                    w_sb[name] = [
                        wpool.tile([128, wid], F32R, tag=f"w{name}{i}", name=f"w{name}{i}") for i in range(KC)
                    ]
                    for i in range(KC):
                        nc.sync.dma_start(
                            out=w_sb[name][i], in_=wdram[i * 128:(i + 1) * 128, :]
                        )

                # qT / kT: weights stationary, activations streamed
                for name, xdram, outT in (("q", xq, qT), ("k", xk, kT)):
                    xt = [xpool.tile([128, S], F32R, tag="x", name="x") for _ in range(KC)]
                    for i in range(KC):
                        nc.sync.dma_start(out=xt[i], in_=xdram[i * 128:(i + 1) * 128, :])
                    for m in range(4):
                        for n in range(S // 512):
                            ps = pjpool.tile([128, 512], F32, tag="pj")
                            for kc in range(KC):
                                nc.tensor.matmul(
                                    ps,
                                    lhsT=w_sb[name][kc][:, m * 128:(m + 1) * 128],
                                    rhs=xt[kc][:, n * 512:(n + 1) * 512],
                                    start=(kc == 0),
                                    stop=(kc == KC - 1),
                                )
                            nc.vector.tensor_copy(outT[m][:, n * 512:(n + 1) * 512], ps)

                # v: activations stationary, weights streamed -> natural layout
                xt = [xpool.tile([128, S], F32R, tag="x", name="x") for _ in range(KC)]
                for i in range(KC):
                    nc.sync.dma_start(out=xt[i], in_=xv[i * 128:(i + 1) * 128, :])
                vchunks = [(0, 512), (512, DPV - 512)]
                for mt in range(S // 128):
                    ps = pjpool.tile([128, DPV], F32, tag="pj", name="psv")
                    for c0, cw in vchunks:
                        for kc in range(KC):
                            nc.tensor.matmul(
                                ps[:, c0:c0 + cw],
                                lhsT=xt[kc][:, mt * 128:(mt + 1) * 128],
                                rhs=w_sb["v"][kc][:, c0:c0 + cw],
                                start=(kc == 0),
                                stop=(kc == KC - 1),
                            )
                    nc.vector.tensor_copy(vp[mt], ps)

            # ---- attention heads ----
            with (
                tc.tile_pool(name="stripes", bufs=stripe_bufs) as stp,
                tc.tile_pool(name="misc", bufs=2) as misc,
                tc.tile_pool(name="dscratch", bufs=4, space="DRAM") as dsc_pool,
                tc.tile_pool(name="spsum", bufs=2, space="PSUM") as sps_pool,
                tc.tile_pool(name="cpsum", bufs=2, space="PSUM") as cps_pool,
            ):
                # heads processed in even/odd pairs: their score matmuls use
                # base partitions 0 and 64 -> distinct PE row-groups -> the
                # two matmuls run concurrently in the array
                for hp in range(H // 2):
                    t = hp
                    heads = (2 * hp, 2 * hp + 1)
                    for blk in range(NB):
                        q0 = blk * BQ
                        cpss = [
                            cps_pool.tile([65, BQ], F32, tag="c", name="cps")
                            for _ in heads
                        ]
                        pts = {0: [], 1: []}
                        for j in range(NSK):
                            spss = [
                                sps_pool.tile([128, BQ], F32, tag="s", name="sps")
                                for _ in heads
                            ]
                            for n in range(NQC):
                                for hi in range(2):
                                    off = hi * 64
                                    nc.tensor.matmul(
                                        spss[hi][:, n * 512:(n + 1) * 512],
                                        lhsT=kT[t][off:off + DH, j * 128:(j + 1) * 128],
                                        rhs=qT[t][off:off + DH, q0 + n * 512:q0 + (n + 1) * 512],
                                        start=True,
                                        stop=True,
                                    )
                            for hi, h in enumerate(heads):
                                pt = stp.tile([128, BQ], F16, tag="pt")
                                nc.scalar.activation(
                                    pt, spss[hi], mybir.ActivationFunctionType.Exp
                                )
                                for n in range(NQC):
                                    nc.tensor.matmul(
                                        cpss[hi][0:65, n * 512:(n + 1) * 512],
                                        lhsT=vp[j][:, h * HPV:h * HPV + 65],
                                        rhs=pt[:, n * 512:(n + 1) * 512],
                                        start=(j == 0),
                                        stop=(j == NSK - 1),
                                    )
                                pts[hi].append(pt)
                        for hi, h in enumerate(heads):
                            cps = cpss[hi]
                            # denominator row -> [128, 8] scatter -> cheap
                            # reciprocal -> gather -> PE broadcast
                            drow = misc.tile([1, BQ], F32, tag="drow")
                            nc.vector.tensor_copy(drow, cps[64:65, :])
                            dsq = misc.tile([128, BQ // 128], F32, tag="dsq")
                            nc.sync.dma_start(out=dsq, in_=drow)
                            rsq = misc.tile([128, BQ // 128], F16, tag="rsq")
                            with nc.allow_low_precision(
                                reason="f16 softmax denominators are plenty"
                            ):
                                nc.vector.reciprocal(rsq, dsq)
                            dscr = dsc_pool.tile([BQ], F16, tag="dscr")
                            nc.sync.dma_start(out=dscr, in_=rsq)
                            rb16 = misc.tile([128, BQ], F16, tag="rb16")
                            nc.sync.dma_start(
                                out=rb16,
                                in_=bass.AP(
                                    tensor=dscr.tensor,
                                    offset=dscr.offset,
                                    ap=[[0, 128]] + list(dscr.ap),
                                ),
                            )
                            # context slice for this (head, block)
                            ctx = misc.tile([DH, BQ], F32, tag="ctx")
                            nc.vector.tensor_mul(ctx, cps[0:DH, :], rb16[0:DH, :])
                            nc.sync.dma_start(
                                out=ctxT[h * DH:(h + 1) * DH, q0:q0 + BQ], in_=ctx
                            )
                            # normalize + write out the attention stripes
                            for j in range(NSK):
                                nc.vector.tensor_mul(pts[hi][j], pts[hi][j], rb16)
                                nc.sync.dma_start(
                                    out=attnT[h, j * 128:(j + 1) * 128, q0:q0 + BQ],
                                    in_=pts[hi][j],
                                )
    nc.finalize()
    return nc


def _prep_weights(Wq, bq, Wk, bk, Wv, bv):
    """Padded head-major, bias-augmented weight mats [DIN, DP]."""
    scale = np.float32(1.0 / np.sqrt(np.float32(DH)))
    wqp = np.zeros((DIN, DP), np.float32)
    wkp = np.zeros((DIN, DP), np.float32)
    wvp = np.zeros((DIN, DPV), np.float32)
    for h in range(H):
        src = slice(h * DH, (h + 1) * DH)
        dst = slice(h * HP, h * HP + DH)
        dstv = slice(h * HPV, h * HPV + DH)
        wqp[0:D, dst] = Wq[:, src] * scale
        wqp[D, dst] = bq[src] * scale
        wkp[0:D, dst] = Wk[:, src]
        wkp[D, dst] = bk[src]
        wvp[0:D, dstv] = Wv[:, src]
        wvp[D, dstv] = bv[src]
        wvp[D, h * HPV + 64] = 1.0  # fused ones column -> softmax denominators
    return wqp, wkp, wvp


def _prep_x(x):
    """[S, D] -> transposed+augmented [DIN, S]."""
    S = x.shape[0]
    xa = np.zeros((DIN, S), np.float32)
    xa[0:D] = x.T
    xa[D] = 1.0
    return xa


def run_attention(query, key, value, Wq, bq, Wk, bk, Wv, bv, S, BQ, trace=False,
                  **run_kwargs):
    nb = query.shape[0]
    nc = build_nc(S, BQ)
    wqp, wkp, wvp = _prep_weights(Wq, bq, Wk, bk, Wv, bv)
    in_maps = []
    for b in range(nb):
        in_maps.append(
            {
                "xq": _prep_x(query[b]),
                "xk": _prep_x(key[b]),
                "xv": _prep_x(value[b]),
                "wq": wqp,
                "wk": wkp,
                "wv": wvp,
            }
        )
    res = run_bass_kernel_spmd(
        nc, in_maps, core_ids=list(range(nb)), trace=trace, **run_kwargs
    )
    # gather: context[b] = ctxT_b.T ; attn[h*B+b, sq, sk] = attnT_b[h, sk, sq]
    context = np.stack([res.results[b]["ctxT"].T for b in range(nb)], axis=0)
    attn = (
        np.stack([res.results[b]["attnT"] for b in range(nb)], axis=1)
        .astype(np.float32)
        .reshape(H * nb, S, S)
        .swapaxes(1, 2)
    )
    return (context, attn), res


def kernel(query, key, value, Wq, bq, Wk, bk, Wv, bv):
    query = np.asarray(query, np.float32)
    key = np.asarray(key, np.float32)
    value = np.asarray(value, np.float32)
    Wq = np.asarray(Wq, np.float32)
    bq = np.asarray(bq, np.float32)
    Wk = np.asarray(Wk, np.float32)
    bk = np.asarray(bk, np.float32)
    Wv = np.asarray(Wv, np.float32)
    bv = np.asarray(bv, np.float32)
    S = query.shape[1]
    out, _ = run_attention(
        query, key, value, Wq, bq, Wk, bk, Wv, bv, S=S, BQ=1024
    )
    return out
